# revision 2
# baseline (speedup 1.0000x reference)
"""AtlasMAG block: host glue + Bass SPMD device kernels on 8 TRN2 NeuronCores.

Device offload (tensor-parallel over 8 cores, AllReduce epilogue):
  1. memory-branch MLP:  silu(featT.T @ w1_shard) @ w2_shard   (B*S x 4160 x 2048 x 1024)
  2. gated FFN:          (silu(h2@w1_s) * (h2@w3_s)) @ w2_s    (B*S x 1024 x 2730 x 1024)
Host (numpy): rmsnorm/qkv/rope/gamma/cummean/attention/TTL grads/Newton-Schulz.
"""
import sys

sys.path.insert(0, "/opt/trn_rl_repo")

import numpy as np
import ml_dtypes

import concourse.bass as bass
import concourse.bacc as bacc
import concourse.mybir as mybir
import concourse.tile as tile
from concourse.bass_utils import run_bass_kernel_spmd

BF16 = ml_dtypes.bfloat16
N_CORES = 8
B, S, D, H = 2, 2048, 1024, 16
HD = D // H
F_POLY = HD + HD * HD            # 4160
M_HID = 2 * D                    # 2048
FFN_H = int(D * 4 * 2 / 3)       # 2730
FFN_H_PAD = 3072                 # 8 * 384
TOK = B * S                      # 4096
EPS = 1e-6
OMEGA_W, OMEGA_DECAY = 64, 0.95
TTL_ALPHA, TTL_ETA, NS_ITERS = 0.999, 0.01, 5

TRACE = False
EXEC_TIMES_NS = []

_GRAPH_CACHE = {}


def _ceil_chunks(total, c=128):
    out = []
    s = 0
    while s < total:
        out.append((s, min(c, total - s)))
        s += c
    return out


def _build_tp_mlp(K, n_shard, gated):
    """out(1024, TOK) = AllReduce_c[ w2_c.T @ act( w1_c.T @ xT ) ]
    act = silu, or silu(a1)*a3 when gated."""
    nc = bacc.Bacc("TRN2", target_bir_lowering=False, debug=False,
                   num_devices=N_CORES)
    bf = mybir.dt.bfloat16
    f32 = mybir.dt.float32
    xT = nc.declare_dram_parameter("xT", [K, TOK], bf, isOutput=False)
    w1 = nc.declare_dram_parameter("w1", [K, n_shard], bf, isOutput=False)
    if gated:
        w3 = nc.declare_dram_parameter("w3", [K, n_shard], bf, isOutput=False)
    w2 = nc.declare_dram_parameter("w2", [n_shard, D], bf, isOutput=False)
    out = nc.declare_dram_parameter("out", [D, TOK], f32, isOutput=True)

    kch = _ceil_chunks(K)          # input-feature chunks (<=128)
    n_m = n_shard // 128           # hidden tiles per shard
    TT = 512                       # token tile
    n_t = TOK // TT

    with tile.TileContext(nc) as tc:
        with tc.tile_pool(name="wp", bufs=1) as wp, \
             tc.tile_pool(name="xp", bufs=2) as xp, \
             tc.tile_pool(name="gp", bufs=2 * n_m + 2) as gp, \
             tc.tile_pool(name="sp", bufs=3) as spool, \
             tc.tile_pool(name="op", bufs=3) as op, \
             tc.tile_pool(name="ps", bufs=2, space="PSUM") as ps, \
             tc.tile_pool(name="dram", bufs=1, space="DRAM") as dram:
            in_b = dram.tile([D, TOK], f32)
            out_b = dram.tile([D, TOK], f32, addr_space="Shared")

            # resident weights
            w1s = []
            w3s = []
            for (s0, c) in kch:
                t = wp.tile([c, n_shard], bf, tag=f"w1_{s0}")
                nc.sync.dma_start(t[:], w1[s0:s0 + c, :])
                w1s.append(t)
                if gated:
                    t3 = wp.tile([c, n_shard], bf, tag=f"w3_{s0}")
                    nc.sync.dma_start(t3[:], w3[s0:s0 + c, :])
                    w3s.append(t3)
            w2s = []
            for mi in range(n_m):
                t = wp.tile([128, D], bf, tag=f"w2_{mi}")
                nc.sync.dma_start(t[:], w2[mi * 128:(mi + 1) * 128, :])
                w2s.append(t)

            for ti in range(n_t):
                t0 = ti * TT
                xts = []
                for (s0, c) in kch:
                    xt = xp.tile([c, TT], bf, tag=f"x_{s0}")
                    nc.sync.dma_start(xt[:], xT[s0:s0 + c, t0:t0 + TT])
                    xts.append(xt)
                gts = []
                for mi in range(n_m):
                    a1 = ps.tile([128, TT], f32, tag="a1")
                    for ki, (s0, c) in enumerate(kch):
                        nc.tensor.matmul(
                            a1[:], w1s[ki][:, mi * 128:(mi + 1) * 128], xts[ki][:],
                            start=(ki == 0), stop=(ki == len(kch) - 1))
                    g = gp.tile([128, TT], bf, tag=f"g_{mi}")
                    if gated:
                        a3 = ps.tile([128, TT], f32, tag="a3")
                        for ki, (s0, c) in enumerate(kch):
                            nc.tensor.matmul(
                                a3[:], w3s[ki][:, mi * 128:(mi + 1) * 128], xts[ki][:],
                                start=(ki == 0), stop=(ki == len(kch) - 1))
                        s1 = spool.tile([128, TT], f32, tag="s1")
                        nc.scalar.activation(s1[:], a1[:],
                                             mybir.ActivationFunctionType.Silu)
                        nc.vector.tensor_mul(g[:], s1[:], a3[:])
                    else:
                        nc.scalar.activation(g[:], a1[:],
                                             mybir.ActivationFunctionType.Silu)
                    gts.append(g)
                for mo in range(D // 128):
                    po = ps.tile([128, TT], f32, tag="po")
                    for ki in range(n_m):
                        nc.tensor.matmul(
                            po[:], w2s[ki][:, mo * 128:(mo + 1) * 128], gts[ki][:],
                            start=(ki == 0), stop=(ki == n_m - 1))
                    oc = op.tile([128, TT], f32, tag="oc")
                    nc.vector.tensor_copy(oc[:], po[:])
                    nc.sync.dma_start(
                        in_b[mo * 128:(mo + 1) * 128, t0:t0 + TT], oc[:])

            nc.gpsimd.collective_compute(
                "AllReduce", mybir.AluOpType.add,
                replica_groups=[list(range(N_CORES))],
                ins=[in_b[:]], outs=[out_b[:]])
            nc.sync.dma_start(out[:], out_b[:])
    nc.compile()
    return nc


def _run_tp(nc, in_maps):
    res = run_bass_kernel_spmd(nc, in_maps, list(range(N_CORES)), trace=TRACE)
    if res.exec_time_ns is not None:
        EXEC_TIMES_NS.append(res.exec_time_ns)
    return res.results[0]["out"]


def _mem_mlp_device(featT_bf, w1u, w2u):
    key = ("mem", F_POLY, M_HID // N_CORES, False)
    if key not in _GRAPH_CACHE:
        _GRAPH_CACHE[key] = _build_tp_mlp(F_POLY, M_HID // N_CORES, False)
    nc = _GRAPH_CACHE[key]
    ns = M_HID // N_CORES
    in_maps = []
    for c in range(N_CORES):
        in_maps.append({
            "xT": featT_bf,
            "w1": np.ascontiguousarray(w1u[:, c * ns:(c + 1) * ns]).astype(BF16),
            "w2": np.ascontiguousarray(w2u[c * ns:(c + 1) * ns, :]).astype(BF16),
        })
    return _run_tp(nc, in_maps)


def _ffn_device(h2T_bf, fw1, fw2, fw3):
    key = ("ffn", D, FFN_H_PAD // N_CORES, True)
    if key not in _GRAPH_CACHE:
        _GRAPH_CACHE[key] = _build_tp_mlp(D, FFN_H_PAD // N_CORES, True)
    nc = _GRAPH_CACHE[key]
    ns = FFN_H_PAD // N_CORES
    w1p = np.zeros((D, FFN_H_PAD), np.float32)
    w1p[:, :FFN_H] = fw1
    w3p = np.zeros((D, FFN_H_PAD), np.float32)
    w3p[:, :FFN_H] = fw3
    w2p = np.zeros((FFN_H_PAD, D), np.float32)
    w2p[:FFN_H, :] = fw2
    in_maps = []
    for c in range(N_CORES):
        in_maps.append({
            "xT": h2T_bf,
            "w1": np.ascontiguousarray(w1p[:, c * ns:(c + 1) * ns]).astype(BF16),
            "w3": np.ascontiguousarray(w3p[:, c * ns:(c + 1) * ns]).astype(BF16),
            "w2": np.ascontiguousarray(w2p[c * ns:(c + 1) * ns, :]).astype(BF16),
        })
    return _run_tp(nc, in_maps)


# ---------------- host math ----------------

def _rmsnorm(x, w):
    return x * (1.0 / np.sqrt(np.mean(x * x, -1, keepdims=True) + EPS)) * w


def _sigmoid(x):
    return 1.0 / (1.0 + np.exp(-x))


def _silu(x):
    return x * _sigmoid(x)


def _rope(q, k):
    half = HD // 2
    inv = 1.0 / (10000.0 ** (np.arange(half, dtype=np.float32) / half))
    fr = np.arange(S, dtype=np.float32)[:, None] * inv[None, :]
    cos, sin = np.cos(fr), np.sin(fr)

    def rot(x):
        x1, x2 = x[..., :half], x[..., half:]
        return np.concatenate([x1 * cos - x2 * sin, x1 * sin + x2 * cos], -1)

    return rot(q), rot(k)


def _phi2(z):
    outer = (z[..., :, None] * z[..., None, :]).reshape(*z.shape[:-1], HD * HD)
    return np.concatenate([z, outer / np.sqrt(np.float32(HD))], -1)


def _newton_schulz(G):
    a, b, c = 3.4445, -4.7750, 2.0315
    X = (G / (np.linalg.norm(G) + 1e-7)).astype(np.float32)
    tall = X.shape[0] > X.shape[1]
    X = X.T if tall else X
    for _ in range(NS_ITERS):
        A = X @ X.T
        X = a * X + (b * A + c * (A @ A)) @ X
    return X.T if tall else X


def kernel(x, norm1_w, norm2_w, qkv_w, q_norm_w, k_norm_w, gamma_w1, gamma_w2,
           mem_wk, mem_w1, mem_w2, memory_gate, wo_w, ffn_w1, ffn_w2, ffn_w3):
    x = np.asarray(x, np.float32)
    f32 = np.float32

    h = _rmsnorm(x, np.asarray(norm1_w, f32))
    qkv = h.reshape(TOK, D) @ np.asarray(qkv_w, f32)
    q, k, v = np.split(qkv.reshape(B, S, 3 * D), 3, axis=-1)

    def heads(t):
        return t.reshape(B, S, H, HD).transpose(0, 2, 1, 3)

    q, k, v = heads(q), heads(k), heads(v)
    q = _rmsnorm(q, np.asarray(q_norm_w, f32))
    k = _rmsnorm(k, np.asarray(k_norm_w, f32))
    q, k = _rope(q, k)

    gamma = _sigmoid(_silu(h @ np.asarray(gamma_w1, f32)) @ np.asarray(gamma_w2, f32))

    k_cummean = np.cumsum(k, axis=2) / np.arange(1, S + 1, dtype=f32)[None, None, :, None]
    g = gamma[:, None, :, :]
    q_mem = g * q + (1.0 - g) * k_cummean
    q_mem_flat = q_mem.transpose(0, 2, 1, 3).reshape(B, S, D)
    v_flat = v.transpose(0, 2, 1, 3).reshape(B, S, D)

    # ---- TTL grads (w_omega nonzero only on last OMEGA_W positions) ----
    mem_wk = np.asarray(mem_wk, f32)
    mem_w1 = np.asarray(mem_w1, f32)
    mem_w2 = np.asarray(mem_w2, f32)
    pos = np.arange(S)
    dpow = (np.float32(OMEGA_DECAY) ** (S - 1 - pos).astype(f32)).astype(f32)
    dpow = np.where(pos >= S - OMEGA_W, dpow, 0.0).astype(f32)
    w_omega = gamma[..., 0] * dpow                     # (B,S)
    denom = np.sum(w_omega) + 1e-8

    T0 = S - OMEGA_W
    qm_t = q_mem_flat[:, T0:]                          # (B,64,D)
    v_t = v_flat[:, T0:]
    z_t = qm_t @ mem_wk                                # (B,64,HD)
    ft = _phi2(z_t)                                    # (B,64,F_POLY)
    a1 = ft @ mem_w1
    sg = _sigmoid(a1)
    h1 = a1 * sg
    pred = h1 @ mem_w2
    diff = pred - v_t
    dpred = (2.0 / denom) * w_omega[:, T0:, None] * diff
    g2 = np.einsum('btm,btd->md', h1, dpred).astype(f32)
    da1 = (dpred @ mem_w2.T) * (sg * (1.0 + a1 * (1.0 - sg)))
    g1 = np.einsum('btf,btm->fm', ft, da1).astype(f32)
    dfeat = da1 @ mem_w1.T
    dz = dfeat[..., :HD].copy()
    dO = dfeat[..., HD:].reshape(B, OMEGA_W, HD, HD)
    dz += np.einsum('btij,btj->bti', dO + dO.transpose(0, 1, 3, 2),
                    z_t) / np.sqrt(np.float32(HD))
    gk = np.einsum('btd,bte->de', qm_t, dz).astype(f32)

    mem_wk_u = TTL_ALPHA * mem_wk - TTL_ETA * _newton_schulz(gk)
    mem_w1_u = TTL_ALPHA * mem_w1 - TTL_ETA * _newton_schulz(g1)
    mem_w2_u = TTL_ALPHA * mem_w2 - TTL_ETA * _newton_schulz(g2)

    # ---- memory branch forward on device ----
    z_full = q_mem_flat @ mem_wk_u                      # (B,S,HD)
    feat = _phi2(z_full).reshape(TOK, F_POLY)
    featT_bf = np.ascontiguousarray(feat.T).astype(BF16)
    mem_T = _mem_mlp_device(featT_bf, mem_w1_u, mem_w2_u)   # (D, TOK) f32
    mem_out = mem_T.T.reshape(B, S, D) * _sigmoid(np.asarray(memory_gate, f32))

    # ---- attention on host ----
    scale = HD ** -0.5
    attn_out = np.empty((B, H, S, HD), f32)
    causal_bias = np.triu(np.full((S, S), -np.inf, f32), 1)
    for b in range(B):
        for hh in range(H):
            sc = (q[b, hh] @ k[b, hh].T) * scale + causal_bias
            sc -= sc.max(-1, keepdims=True)
            e = np.exp(sc)
            p = e / e.sum(-1, keepdims=True)
            attn_out[b, hh] = p @ v[b, hh]
    attn_out = attn_out.transpose(0, 2, 1, 3).reshape(B, S, D) @ np.asarray(wo_w, f32)

    x_mid = x + attn_out + mem_out
    h2 = _rmsnorm(x_mid, np.asarray(norm2_w, f32))
    h2T_bf = np.ascontiguousarray(h2.reshape(TOK, D).T).astype(BF16)
    y_T = _ffn_device(h2T_bf, np.asarray(ffn_w1, f32), np.asarray(ffn_w2, f32),
                      np.asarray(ffn_w3, f32))         # (D, TOK)
    out = x_mid + y_T.T.reshape(B, S, D)
    return out.astype(np.float32)


# revision 5
# speedup vs baseline: 1.4054x; 1.4054x over previous
"""AtlasMAG block: host glue + Bass SPMD device kernels on 8 TRN2 NeuronCores.

Device offload (tensor-parallel over 8 cores, AllReduce epilogue):
  1. memory-branch MLP:  silu(featT.T @ w1_shard) @ w2_shard   (B*S x 4160 x 2048 x 1024)
  2. gated FFN:          (silu(h2@w1_s) * (h2@w3_s)) @ w2_s    (B*S x 1024 x 2730 x 1024)
Host (numpy): rmsnorm/qkv/rope/gamma/cummean/attention/TTL grads/Newton-Schulz.
"""
import sys

sys.path.insert(0, "/opt/trn_rl_repo")

import numpy as np
import ml_dtypes

import concourse.bass as bass
import concourse.bacc as bacc
import concourse.mybir as mybir
import concourse.tile as tile
from concourse.bass_utils import run_bass_kernel_spmd

BF16 = ml_dtypes.bfloat16
N_CORES = 8
B, S, D, H = 2, 2048, 1024, 16
HD = D // H
F_POLY = HD + HD * HD            # 4160
M_HID = 2 * D                    # 2048
FFN_H = int(D * 4 * 2 / 3)       # 2730
FFN_H_PAD = 3072                 # 8 * 384
TOK = B * S                      # 4096
EPS = 1e-6
OMEGA_W, OMEGA_DECAY = 64, 0.95
TTL_ALPHA, TTL_ETA, NS_ITERS = 0.999, 0.01, 5

TRACE = False
EXEC_TIMES_NS = []

_GRAPH_CACHE = {}


def _ceil_chunks(total, c=128):
    out = []
    s = 0
    while s < total:
        out.append((s, min(c, total - s)))
        s += c
    return out


def _build_tp_mlp(K, n_shard, gated):
    """out(1024, TOK) = AllReduce_c[ w2_c.T @ act( w1_c.T @ xT ) ]
    act = silu, or silu(a1)*a3 when gated."""
    nc = bacc.Bacc("TRN2", target_bir_lowering=False, debug=False,
                   num_devices=N_CORES)
    bf = mybir.dt.bfloat16
    f32 = mybir.dt.float32
    xT = nc.declare_dram_parameter("xT", [K, TOK], bf, isOutput=False)
    w1 = nc.declare_dram_parameter("w1", [K, n_shard], bf, isOutput=False)
    if gated:
        w3 = nc.declare_dram_parameter("w3", [K, n_shard], bf, isOutput=False)
    w2 = nc.declare_dram_parameter("w2", [n_shard, D], bf, isOutput=False)
    out = nc.declare_dram_parameter("out", [D, TOK], bf, isOutput=True)

    kch = _ceil_chunks(K)          # input-feature chunks (<=128)
    n_m = n_shard // 128           # hidden tiles per shard
    TT = 512                       # token tile
    n_t = TOK // TT

    with tile.TileContext(nc) as tc:
        with tc.tile_pool(name="wp", bufs=1) as wp, \
             tc.tile_pool(name="xp", bufs=3) as xp, \
             tc.tile_pool(name="gp", bufs=2 * n_m + 2) as gp, \
             tc.tile_pool(name="sp", bufs=3) as spool, \
             tc.tile_pool(name="op", bufs=3) as op, \
             tc.tile_pool(name="ps", bufs=2, space="PSUM") as ps, \
             tc.tile_pool(name="dram", bufs=1, space="DRAM") as dram:
            in_bs = [dram.tile([D, TT], bf, tag=f"inb_{i}", name=f"inb_{i}")
                     for i in range(n_t)]
            out_bs = [dram.tile([D, TT], bf, tag=f"outb_{i}", name=f"outb_{i}", addr_space="Shared")
                      for i in range(n_t)]

            # resident weights
            w1s = []
            w3s = []
            for (s0, c) in kch:
                t = wp.tile([c, n_shard], bf, tag=f"w1_{s0}")
                nc.sync.dma_start(t[:], w1[s0:s0 + c, :])
                w1s.append(t)
                if gated:
                    t3 = wp.tile([c, n_shard], bf, tag=f"w3_{s0}")
                    nc.sync.dma_start(t3[:], w3[s0:s0 + c, :])
                    w3s.append(t3)
            w2s = []
            for mi in range(n_m):
                t = wp.tile([128, D], bf, tag=f"w2_{mi}")
                nc.sync.dma_start(t[:], w2[mi * 128:(mi + 1) * 128, :])
                w2s.append(t)

            for ti in range(n_t):
                t0 = ti * TT
                xts = []
                for (s0, c) in kch:
                    xt = xp.tile([c, TT], bf, tag=f"x_{s0}")
                    nc.sync.dma_start(xt[:], xT[s0:s0 + c, t0:t0 + TT])
                    xts.append(xt)
                gts = []
                for mi in range(n_m):
                    a1 = ps.tile([128, TT], f32, tag="a1")
                    for ki, (s0, c) in enumerate(kch):
                        nc.tensor.matmul(
                            a1[:], w1s[ki][:, mi * 128:(mi + 1) * 128], xts[ki][:],
                            start=(ki == 0), stop=(ki == len(kch) - 1))
                    g = gp.tile([128, TT], bf, tag=f"g_{mi}")
                    if gated:
                        a3 = ps.tile([128, TT], f32, tag="a3")
                        for ki, (s0, c) in enumerate(kch):
                            nc.tensor.matmul(
                                a3[:], w3s[ki][:, mi * 128:(mi + 1) * 128], xts[ki][:],
                                start=(ki == 0), stop=(ki == len(kch) - 1))
                        s1 = spool.tile([128, TT], f32, tag="s1")
                        nc.scalar.activation(s1[:], a1[:],
                                             mybir.ActivationFunctionType.Silu)
                        nc.vector.tensor_mul(g[:], s1[:], a3[:])
                    else:
                        nc.scalar.activation(g[:], a1[:],
                                             mybir.ActivationFunctionType.Silu)
                    gts.append(g)
                for mo in range(D // 128):
                    po = ps.tile([128, TT], f32, tag="po")
                    for ki in range(n_m):
                        nc.tensor.matmul(
                            po[:], w2s[ki][:, mo * 128:(mo + 1) * 128], gts[ki][:],
                            start=(ki == 0), stop=(ki == n_m - 1))
                    oc = op.tile([128, TT], bf, tag="oc")
                    nc.vector.tensor_copy(oc[:], po[:])
                    nc.sync.dma_start(
                        in_bs[ti][mo * 128:(mo + 1) * 128, :], oc[:])

                nc.gpsimd.collective_compute(
                    "AllReduce", mybir.AluOpType.add,
                    replica_groups=[list(range(N_CORES))],
                    ins=[in_bs[ti][:]], outs=[out_bs[ti][:]])
                nc.sync.dma_start(out[:, t0:t0 + TT], out_bs[ti][:])
    nc.compile()
    return nc


def _run_tp(nc, in_maps):
    res = run_bass_kernel_spmd(nc, in_maps, list(range(N_CORES)), trace=TRACE)
    if res.exec_time_ns is not None:
        EXEC_TIMES_NS.append(res.exec_time_ns)
    return np.asarray(res.results[0]["out"]).astype(np.float32)


def _mem_mlp_device(featT_bf, w1u, w2u):
    key = ("mem", F_POLY, M_HID // N_CORES, False)
    if key not in _GRAPH_CACHE:
        _GRAPH_CACHE[key] = _build_tp_mlp(F_POLY, M_HID // N_CORES, False)
    nc = _GRAPH_CACHE[key]
    ns = M_HID // N_CORES
    in_maps = []
    for c in range(N_CORES):
        in_maps.append({
            "xT": featT_bf,
            "w1": np.ascontiguousarray(w1u[:, c * ns:(c + 1) * ns]).astype(BF16),
            "w2": np.ascontiguousarray(w2u[c * ns:(c + 1) * ns, :]).astype(BF16),
        })
    return _run_tp(nc, in_maps)


def _ffn_device(h2T_bf, fw1, fw2, fw3):
    key = ("ffn", D, FFN_H_PAD // N_CORES, True)
    if key not in _GRAPH_CACHE:
        _GRAPH_CACHE[key] = _build_tp_mlp(D, FFN_H_PAD // N_CORES, True)
    nc = _GRAPH_CACHE[key]
    ns = FFN_H_PAD // N_CORES
    w1p = np.zeros((D, FFN_H_PAD), np.float32)
    w1p[:, :FFN_H] = fw1
    w3p = np.zeros((D, FFN_H_PAD), np.float32)
    w3p[:, :FFN_H] = fw3
    w2p = np.zeros((FFN_H_PAD, D), np.float32)
    w2p[:FFN_H, :] = fw2
    in_maps = []
    for c in range(N_CORES):
        in_maps.append({
            "xT": h2T_bf,
            "w1": np.ascontiguousarray(w1p[:, c * ns:(c + 1) * ns]).astype(BF16),
            "w3": np.ascontiguousarray(w3p[:, c * ns:(c + 1) * ns]).astype(BF16),
            "w2": np.ascontiguousarray(w2p[c * ns:(c + 1) * ns, :]).astype(BF16),
        })
    return _run_tp(nc, in_maps)


# ---------------- host math ----------------

def _rmsnorm(x, w):
    return x * (1.0 / np.sqrt(np.mean(x * x, -1, keepdims=True) + EPS)) * w


def _sigmoid(x):
    return 1.0 / (1.0 + np.exp(-x))


def _silu(x):
    return x * _sigmoid(x)


def _rope(q, k):
    half = HD // 2
    inv = 1.0 / (10000.0 ** (np.arange(half, dtype=np.float32) / half))
    fr = np.arange(S, dtype=np.float32)[:, None] * inv[None, :]
    cos, sin = np.cos(fr), np.sin(fr)

    def rot(x):
        x1, x2 = x[..., :half], x[..., half:]
        return np.concatenate([x1 * cos - x2 * sin, x1 * sin + x2 * cos], -1)

    return rot(q), rot(k)


def _phi2(z):
    outer = (z[..., :, None] * z[..., None, :]).reshape(*z.shape[:-1], HD * HD)
    return np.concatenate([z, outer / np.sqrt(np.float32(HD))], -1)


def _newton_schulz(G):
    a, b, c = 3.4445, -4.7750, 2.0315
    X = (G / (np.linalg.norm(G) + 1e-7)).astype(np.float32)
    tall = X.shape[0] > X.shape[1]
    X = X.T if tall else X
    for _ in range(NS_ITERS):
        A = X @ X.T
        X = a * X + (b * A + c * (A @ A)) @ X
    return X.T if tall else X


def kernel(x, norm1_w, norm2_w, qkv_w, q_norm_w, k_norm_w, gamma_w1, gamma_w2,
           mem_wk, mem_w1, mem_w2, memory_gate, wo_w, ffn_w1, ffn_w2, ffn_w3):
    x = np.asarray(x, np.float32)
    f32 = np.float32

    h = _rmsnorm(x, np.asarray(norm1_w, f32))
    qkv = h.reshape(TOK, D) @ np.asarray(qkv_w, f32)
    q, k, v = np.split(qkv.reshape(B, S, 3 * D), 3, axis=-1)

    def heads(t):
        return t.reshape(B, S, H, HD).transpose(0, 2, 1, 3)

    q, k, v = heads(q), heads(k), heads(v)
    q = _rmsnorm(q, np.asarray(q_norm_w, f32))
    k = _rmsnorm(k, np.asarray(k_norm_w, f32))
    q, k = _rope(q, k)

    gamma = _sigmoid(_silu(h @ np.asarray(gamma_w1, f32)) @ np.asarray(gamma_w2, f32))

    k_cummean = np.cumsum(k, axis=2) / np.arange(1, S + 1, dtype=f32)[None, None, :, None]
    g = gamma[:, None, :, :]
    q_mem = g * q + (1.0 - g) * k_cummean
    q_mem_flat = q_mem.transpose(0, 2, 1, 3).reshape(B, S, D)
    v_flat = v.transpose(0, 2, 1, 3).reshape(B, S, D)

    # ---- TTL grads (w_omega nonzero only on last OMEGA_W positions) ----
    mem_wk = np.asarray(mem_wk, f32)
    mem_w1 = np.asarray(mem_w1, f32)
    mem_w2 = np.asarray(mem_w2, f32)
    pos = np.arange(S)
    dpow = (np.float32(OMEGA_DECAY) ** (S - 1 - pos).astype(f32)).astype(f32)
    dpow = np.where(pos >= S - OMEGA_W, dpow, 0.0).astype(f32)
    w_omega = gamma[..., 0] * dpow                     # (B,S)
    denom = np.sum(w_omega) + 1e-8

    T0 = S - OMEGA_W
    qm_t = q_mem_flat[:, T0:]                          # (B,64,D)
    v_t = v_flat[:, T0:]
    z_t = qm_t @ mem_wk                                # (B,64,HD)
    ft = _phi2(z_t)                                    # (B,64,F_POLY)
    a1 = ft @ mem_w1
    sg = _sigmoid(a1)
    h1 = a1 * sg
    pred = h1 @ mem_w2
    diff = pred - v_t
    dpred = (2.0 / denom) * w_omega[:, T0:, None] * diff
    g2 = np.einsum('btm,btd->md', h1, dpred).astype(f32)
    da1 = (dpred @ mem_w2.T) * (sg * (1.0 + a1 * (1.0 - sg)))
    g1 = np.einsum('btf,btm->fm', ft, da1).astype(f32)
    dfeat = da1 @ mem_w1.T
    dz = dfeat[..., :HD].copy()
    dO = dfeat[..., HD:].reshape(B, OMEGA_W, HD, HD)
    dz += np.einsum('btij,btj->bti', dO + dO.transpose(0, 1, 3, 2),
                    z_t) / np.sqrt(np.float32(HD))
    gk = np.einsum('btd,bte->de', qm_t, dz).astype(f32)

    mem_wk_u = TTL_ALPHA * mem_wk - TTL_ETA * _newton_schulz(gk)
    mem_w1_u = TTL_ALPHA * mem_w1 - TTL_ETA * _newton_schulz(g1)
    mem_w2_u = TTL_ALPHA * mem_w2 - TTL_ETA * _newton_schulz(g2)

    # ---- memory branch forward on device ----
    z_full = q_mem_flat @ mem_wk_u                      # (B,S,HD)
    feat = _phi2(z_full).reshape(TOK, F_POLY)
    featT_bf = np.ascontiguousarray(feat.T).astype(BF16)
    mem_T = _mem_mlp_device(featT_bf, mem_w1_u, mem_w2_u)   # (D, TOK) f32
    mem_out = mem_T.T.reshape(B, S, D) * _sigmoid(np.asarray(memory_gate, f32))

    # ---- attention on host ----
    scale = HD ** -0.5
    attn_out = np.empty((B, H, S, HD), f32)
    causal_bias = np.triu(np.full((S, S), -np.inf, f32), 1)
    for b in range(B):
        for hh in range(H):
            sc = (q[b, hh] @ k[b, hh].T) * scale + causal_bias
            sc -= sc.max(-1, keepdims=True)
            e = np.exp(sc)
            p = e / e.sum(-1, keepdims=True)
            attn_out[b, hh] = p @ v[b, hh]
    attn_out = attn_out.transpose(0, 2, 1, 3).reshape(B, S, D) @ np.asarray(wo_w, f32)

    x_mid = x + attn_out + mem_out
    h2 = _rmsnorm(x_mid, np.asarray(norm2_w, f32))
    h2T_bf = np.ascontiguousarray(h2.reshape(TOK, D).T).astype(BF16)
    y_T = _ffn_device(h2T_bf, np.asarray(ffn_w1, f32), np.asarray(ffn_w2, f32),
                      np.asarray(ffn_w3, f32))         # (D, TOK)
    out = x_mid + y_T.T.reshape(B, S, D)
    return out.astype(np.float32)


# revision 7
# speedup vs baseline: 1.7903x; 1.2739x over previous
"""AtlasMAG block: host glue + Bass SPMD device kernels on 8 TRN2 NeuronCores.

Device offload (tensor-parallel over 8 cores, AllReduce epilogue):
  1. memory-branch MLP:  silu(featT.T @ w1_shard) @ w2_shard   (B*S x 4160 x 2048 x 1024)
  2. gated FFN:          (silu(h2@w1_s) * (h2@w3_s)) @ w2_s    (B*S x 1024 x 2730 x 1024)
Host (numpy): rmsnorm/qkv/rope/gamma/cummean/attention/TTL grads/Newton-Schulz.
"""
import sys

sys.path.insert(0, "/opt/trn_rl_repo")

import numpy as np
import ml_dtypes

import concourse.bass as bass
import concourse.bacc as bacc
import concourse.mybir as mybir
import concourse.tile as tile
from concourse.bass_utils import run_bass_kernel_spmd

BF16 = ml_dtypes.bfloat16
N_CORES = 8
B, S, D, H = 2, 2048, 1024, 16
HD = D // H
F_POLY = HD + HD * HD            # 4160
M_HID = 2 * D                    # 2048
FFN_H = int(D * 4 * 2 / 3)       # 2730
FFN_H_PAD = 3072                 # 8 * 384
TOK = B * S                      # 4096
EPS = 1e-6
OMEGA_W, OMEGA_DECAY = 64, 0.95
TTL_ALPHA, TTL_ETA, NS_ITERS = 0.999, 0.01, 5

TRACE = False
EXEC_TIMES_NS = []

_GRAPH_CACHE = {}


def _ceil_chunks(total, c=128):
    out = []
    s = 0
    while s < total:
        out.append((s, min(c, total - s)))
        s += c
    return out


def _build_tp_mlp(K, n_shard, gated):
    """out(1024, TOK) = AllReduce_c[ w2_c.T @ act( w1_c.T @ xT ) ]
    act = silu, or silu(a1)*a3 when gated."""
    nc = bacc.Bacc("TRN2", target_bir_lowering=False, debug=False,
                   num_devices=N_CORES)
    bf = mybir.dt.bfloat16
    f32 = mybir.dt.float32
    xT = nc.declare_dram_parameter("xT", [K, TOK], bf, isOutput=False)
    w1 = nc.declare_dram_parameter("w1", [K, n_shard], bf, isOutput=False)
    if gated:
        w3 = nc.declare_dram_parameter("w3", [K, n_shard], bf, isOutput=False)
    w2 = nc.declare_dram_parameter("w2", [n_shard, D], bf, isOutput=False)
    out = nc.declare_dram_parameter("out", [D // N_CORES, TOK], bf, isOutput=True)

    kch = _ceil_chunks(K)          # input-feature chunks (<=128)
    n_m = n_shard // 128           # hidden tiles per shard
    TT = 512                       # token tile
    n_t = TOK // TT

    with tile.TileContext(nc) as tc:
        with tc.tile_pool(name="wp", bufs=1) as wp, \
             tc.tile_pool(name="xp", bufs=3) as xp, \
             tc.tile_pool(name="gp", bufs=2 * n_m + 2) as gp, \
             tc.tile_pool(name="sp", bufs=3) as spool, \
             tc.tile_pool(name="op", bufs=3) as op, \
             tc.tile_pool(name="ps", bufs=2, space="PSUM") as ps, \
             tc.tile_pool(name="dram", bufs=1, space="DRAM") as dram:
            in_bs = [dram.tile([D, TT], bf, tag=f"inb_{i}", name=f"inb_{i}")
                     for i in range(n_t)]
            out_bs = [dram.tile([D // N_CORES, TT], bf, tag=f"outb_{i}", name=f"outb_{i}")
                      for i in range(n_t)]

            # resident weights
            w1s = []
            w3s = []
            for (s0, c) in kch:
                t = wp.tile([c, n_shard], bf, tag=f"w1_{s0}")
                nc.sync.dma_start(t[:], w1[s0:s0 + c, :])
                w1s.append(t)
                if gated:
                    t3 = wp.tile([c, n_shard], bf, tag=f"w3_{s0}")
                    nc.sync.dma_start(t3[:], w3[s0:s0 + c, :])
                    w3s.append(t3)
            w2s = []
            for mi in range(n_m):
                t = wp.tile([128, D], bf, tag=f"w2_{mi}")
                nc.sync.dma_start(t[:], w2[mi * 128:(mi + 1) * 128, :])
                w2s.append(t)

            for ti in range(n_t):
                t0 = ti * TT
                xts = []
                for (s0, c) in kch:
                    xt = xp.tile([c, TT], bf, tag=f"x_{s0}")
                    nc.sync.dma_start(xt[:], xT[s0:s0 + c, t0:t0 + TT])
                    xts.append(xt)
                gts = []
                for mi in range(n_m):
                    a1 = ps.tile([128, TT], f32, tag="a1")
                    for ki, (s0, c) in enumerate(kch):
                        nc.tensor.matmul(
                            a1[:], w1s[ki][:, mi * 128:(mi + 1) * 128], xts[ki][:],
                            start=(ki == 0), stop=(ki == len(kch) - 1))
                    g = gp.tile([128, TT], bf, tag=f"g_{mi}")
                    if gated:
                        a3 = ps.tile([128, TT], f32, tag="a3")
                        for ki, (s0, c) in enumerate(kch):
                            nc.tensor.matmul(
                                a3[:], w3s[ki][:, mi * 128:(mi + 1) * 128], xts[ki][:],
                                start=(ki == 0), stop=(ki == len(kch) - 1))
                        s1 = spool.tile([128, TT], f32, tag="s1")
                        nc.scalar.activation(s1[:], a1[:],
                                             mybir.ActivationFunctionType.Silu)
                        nc.vector.tensor_mul(g[:], s1[:], a3[:])
                    else:
                        nc.scalar.activation(g[:], a1[:],
                                             mybir.ActivationFunctionType.Silu)
                    gts.append(g)
                for mo in range(D // 128):
                    po = ps.tile([128, TT], f32, tag="po")
                    for ki in range(n_m):
                        nc.tensor.matmul(
                            po[:], w2s[ki][:, mo * 128:(mo + 1) * 128], gts[ki][:],
                            start=(ki == 0), stop=(ki == n_m - 1))
                    oc = op.tile([128, TT], bf, tag="oc")
                    nc.vector.tensor_copy(oc[:], po[:])
                    nc.sync.dma_start(
                        in_bs[ti][mo * 128:(mo + 1) * 128, :], oc[:])

                nc.gpsimd.collective_compute(
                    "ReduceScatter", mybir.AluOpType.add,
                    replica_groups=[list(range(N_CORES))],
                    ins=[in_bs[ti][:]], outs=[out_bs[ti][:]])
                nc.sync.dma_start(out[:, t0:t0 + TT], out_bs[ti][:])
    nc.compile()
    return nc


def _run_tp(nc, in_maps):
    res = run_bass_kernel_spmd(nc, in_maps, list(range(N_CORES)), trace=TRACE)
    if res.exec_time_ns is not None:
        EXEC_TIMES_NS.append(res.exec_time_ns)
    return np.concatenate(
        [np.asarray(res.results[c]["out"]).astype(np.float32)
         for c in range(N_CORES)], axis=0)


def _mem_mlp_device(featT_bf, w1u, w2u):
    key = ("mem", F_POLY, M_HID // N_CORES, False)
    if key not in _GRAPH_CACHE:
        _GRAPH_CACHE[key] = _build_tp_mlp(F_POLY, M_HID // N_CORES, False)
    nc = _GRAPH_CACHE[key]
    ns = M_HID // N_CORES
    in_maps = []
    for c in range(N_CORES):
        in_maps.append({
            "xT": featT_bf,
            "w1": np.ascontiguousarray(w1u[:, c * ns:(c + 1) * ns]).astype(BF16),
            "w2": np.ascontiguousarray(w2u[c * ns:(c + 1) * ns, :]).astype(BF16),
        })
    return _run_tp(nc, in_maps)


def _ffn_device(h2T_bf, fw1, fw2, fw3):
    key = ("ffn", D, FFN_H_PAD // N_CORES, True)
    if key not in _GRAPH_CACHE:
        _GRAPH_CACHE[key] = _build_tp_mlp(D, FFN_H_PAD // N_CORES, True)
    nc = _GRAPH_CACHE[key]
    ns = FFN_H_PAD // N_CORES
    w1p = np.zeros((D, FFN_H_PAD), np.float32)
    w1p[:, :FFN_H] = fw1
    w3p = np.zeros((D, FFN_H_PAD), np.float32)
    w3p[:, :FFN_H] = fw3
    w2p = np.zeros((FFN_H_PAD, D), np.float32)
    w2p[:FFN_H, :] = fw2
    in_maps = []
    for c in range(N_CORES):
        in_maps.append({
            "xT": h2T_bf,
            "w1": np.ascontiguousarray(w1p[:, c * ns:(c + 1) * ns]).astype(BF16),
            "w3": np.ascontiguousarray(w3p[:, c * ns:(c + 1) * ns]).astype(BF16),
            "w2": np.ascontiguousarray(w2p[c * ns:(c + 1) * ns, :]).astype(BF16),
        })
    return _run_tp(nc, in_maps)


# ---------------- host math ----------------

def _rmsnorm(x, w):
    return x * (1.0 / np.sqrt(np.mean(x * x, -1, keepdims=True) + EPS)) * w


def _sigmoid(x):
    return 1.0 / (1.0 + np.exp(-x))


def _silu(x):
    return x * _sigmoid(x)


def _rope(q, k):
    half = HD // 2
    inv = 1.0 / (10000.0 ** (np.arange(half, dtype=np.float32) / half))
    fr = np.arange(S, dtype=np.float32)[:, None] * inv[None, :]
    cos, sin = np.cos(fr), np.sin(fr)

    def rot(x):
        x1, x2 = x[..., :half], x[..., half:]
        return np.concatenate([x1 * cos - x2 * sin, x1 * sin + x2 * cos], -1)

    return rot(q), rot(k)


def _phi2(z):
    outer = (z[..., :, None] * z[..., None, :]).reshape(*z.shape[:-1], HD * HD)
    return np.concatenate([z, outer / np.sqrt(np.float32(HD))], -1)


def _newton_schulz(G):
    a, b, c = 3.4445, -4.7750, 2.0315
    X = (G / (np.linalg.norm(G) + 1e-7)).astype(np.float32)
    tall = X.shape[0] > X.shape[1]
    X = X.T if tall else X
    for _ in range(NS_ITERS):
        A = X @ X.T
        X = a * X + (b * A + c * (A @ A)) @ X
    return X.T if tall else X


def kernel(x, norm1_w, norm2_w, qkv_w, q_norm_w, k_norm_w, gamma_w1, gamma_w2,
           mem_wk, mem_w1, mem_w2, memory_gate, wo_w, ffn_w1, ffn_w2, ffn_w3):
    x = np.asarray(x, np.float32)
    f32 = np.float32

    h = _rmsnorm(x, np.asarray(norm1_w, f32))
    qkv = h.reshape(TOK, D) @ np.asarray(qkv_w, f32)
    q, k, v = np.split(qkv.reshape(B, S, 3 * D), 3, axis=-1)

    def heads(t):
        return t.reshape(B, S, H, HD).transpose(0, 2, 1, 3)

    q, k, v = heads(q), heads(k), heads(v)
    q = _rmsnorm(q, np.asarray(q_norm_w, f32))
    k = _rmsnorm(k, np.asarray(k_norm_w, f32))
    q, k = _rope(q, k)

    gamma = _sigmoid(_silu(h @ np.asarray(gamma_w1, f32)) @ np.asarray(gamma_w2, f32))

    k_cummean = np.cumsum(k, axis=2) / np.arange(1, S + 1, dtype=f32)[None, None, :, None]
    g = gamma[:, None, :, :]
    q_mem = g * q + (1.0 - g) * k_cummean
    q_mem_flat = q_mem.transpose(0, 2, 1, 3).reshape(B, S, D)
    v_flat = v.transpose(0, 2, 1, 3).reshape(B, S, D)

    # ---- TTL grads (w_omega nonzero only on last OMEGA_W positions) ----
    mem_wk = np.asarray(mem_wk, f32)
    mem_w1 = np.asarray(mem_w1, f32)
    mem_w2 = np.asarray(mem_w2, f32)
    pos = np.arange(S)
    dpow = (np.float32(OMEGA_DECAY) ** (S - 1 - pos).astype(f32)).astype(f32)
    dpow = np.where(pos >= S - OMEGA_W, dpow, 0.0).astype(f32)
    w_omega = gamma[..., 0] * dpow                     # (B,S)
    denom = np.sum(w_omega) + 1e-8

    T0 = S - OMEGA_W
    qm_t = q_mem_flat[:, T0:]                          # (B,64,D)
    v_t = v_flat[:, T0:]
    z_t = qm_t @ mem_wk                                # (B,64,HD)
    ft = _phi2(z_t)                                    # (B,64,F_POLY)
    a1 = ft @ mem_w1
    sg = _sigmoid(a1)
    h1 = a1 * sg
    pred = h1 @ mem_w2
    diff = pred - v_t
    dpred = (2.0 / denom) * w_omega[:, T0:, None] * diff
    g2 = np.einsum('btm,btd->md', h1, dpred).astype(f32)
    da1 = (dpred @ mem_w2.T) * (sg * (1.0 + a1 * (1.0 - sg)))
    g1 = np.einsum('btf,btm->fm', ft, da1).astype(f32)
    dfeat = da1 @ mem_w1.T
    dz = dfeat[..., :HD].copy()
    dO = dfeat[..., HD:].reshape(B, OMEGA_W, HD, HD)
    dz += np.einsum('btij,btj->bti', dO + dO.transpose(0, 1, 3, 2),
                    z_t) / np.sqrt(np.float32(HD))
    gk = np.einsum('btd,bte->de', qm_t, dz).astype(f32)

    mem_wk_u = TTL_ALPHA * mem_wk - TTL_ETA * _newton_schulz(gk)
    mem_w1_u = TTL_ALPHA * mem_w1 - TTL_ETA * _newton_schulz(g1)
    mem_w2_u = TTL_ALPHA * mem_w2 - TTL_ETA * _newton_schulz(g2)

    # ---- memory branch forward on device ----
    z_full = q_mem_flat @ mem_wk_u                      # (B,S,HD)
    feat = _phi2(z_full).reshape(TOK, F_POLY)
    featT_bf = np.ascontiguousarray(feat.T).astype(BF16)
    mem_T = _mem_mlp_device(featT_bf, mem_w1_u, mem_w2_u)   # (D, TOK) f32
    mem_out = mem_T.T.reshape(B, S, D) * _sigmoid(np.asarray(memory_gate, f32))

    # ---- attention on host ----
    scale = HD ** -0.5
    attn_out = np.empty((B, H, S, HD), f32)
    causal_bias = np.triu(np.full((S, S), -np.inf, f32), 1)
    for b in range(B):
        for hh in range(H):
            sc = (q[b, hh] @ k[b, hh].T) * scale + causal_bias
            sc -= sc.max(-1, keepdims=True)
            e = np.exp(sc)
            p = e / e.sum(-1, keepdims=True)
            attn_out[b, hh] = p @ v[b, hh]
    attn_out = attn_out.transpose(0, 2, 1, 3).reshape(B, S, D) @ np.asarray(wo_w, f32)

    x_mid = x + attn_out + mem_out
    h2 = _rmsnorm(x_mid, np.asarray(norm2_w, f32))
    h2T_bf = np.ascontiguousarray(h2.reshape(TOK, D).T).astype(BF16)
    y_T = _ffn_device(h2T_bf, np.asarray(ffn_w1, f32), np.asarray(ffn_w2, f32),
                      np.asarray(ffn_w3, f32))         # (D, TOK)
    out = x_mid + y_T.T.reshape(B, S, D)
    return out.astype(np.float32)
